# revision 12
# baseline (speedup 1.0000x reference)
"""Trainium2 Bass kernel for nn_NER_76012331205088.

Reference computation (per batch sample b):
    u = hidden @ u_w.T + u_b                  # [S, 3H]
    v = hidden @ v_w.T + v_b                  # [S, 3H]
    start_logit = (sigmoid(u) * v) @ o_w.T + o_b          # [S, TAG]
    entity = hidden[entity_start]             # [E, H]
    q = (entity @ q_w.T + q_b)  -> [E, HEADS, D]
    k = (hidden @ k_w.T + k_b)  -> [S, HEADS, D]
    end_logit = mean_h(q_h @ k_h.T) / sqrt(D), masked_fill(~mask, -5e4)

Sharding: pure data-parallel over batch B=16 across 8 cores (2 samples per
core); weights replicated to every core; no collectives — the host slices
inputs and concatenates outputs.

Device layout strategy: every matmul contracts along the SBUF partition
dim, so everything is computed in "transposed" space:
    uT[o, s] = sum_h u_wT[h, o] * xT[h, s]
with xT/weights pre-transposed once on the host (pure layout work, no
FLOPs). Matmuls run as float32r (fp32 bits, single-pass PE mode: 1
cycle/row at N>=256 vs 4 for plain fp32). PSUM accumulates in fp32.

The GLU o-projection accumulates start_logit.T directly in a persistent
[TAG, S] PSUM region across all 18 o-tiles (sigmoid(u)*v tiles are consumed
immediately; the [3H, S] GLU activation never materializes). Head-summed
scores accumulate over the 12 heads in PSUM, with the 1/(HEADS*sqrt(D))
scale folded into the PSUM->SBUF copy.

Biases u_b/v_b/k_b/q_b are applied on-device (fused into ACT ops); o_b and
the attention-mask fill are applied on the host (o_b is a rank-1 post-add,
the mask is elementwise on the output).
"""

import os
import sys
from contextlib import ExitStack

import numpy as np

for _p in (
    "/root/.axon_site",
    "/root/.axon_site/_ro/trn_rl_repo",
    "/root/.axon_site/_ro/pypackages",
    "/opt/trn_rl_repo",
):
    if os.path.isdir(_p) and _p not in sys.path:
        sys.path.append(_p)

import concourse.bass as bass
import concourse.mybir as mybir
import concourse.tile as tile
from concourse.bass_utils import run_bass_kernel_spmd
from concourse.vector_clock import ScopedClock

# ---------------------------------------------------------------- constants
B, S, H, E, TAG, HEADS = 16, 2048, 768, 128, 10, 12
D = H // HEADS          # 64
H3 = 3 * H              # 2304
NCORES = 8
BL = B // NCORES        # 2 samples per core
NCHUNK = 512            # free-dim chunk (one fp32 PSUM bank)
NK = H // 128           # 6 contraction tiles over H
NM = H3 // 128          # 18 output tiles over 3H
NN = S // NCHUNK        # 4 sequence chunks
OUT_SCALE = 1.0 / (HEADS * float(D) ** 0.5)   # mean over heads + /sqrt(D)

F32 = mybir.dt.float32
F32R = mybir.dt.float32r
ACT_F = mybir.ActivationFunctionType

PROFILE = False         # set True (module level) to trace + report HW time
LAST_EXEC_TIME_NS = None


# ---------------------------------------------------- wait-split post-pass
# The walrus build in this container rejects instructions carrying more
# than one sem wait ("Too many sync wait commands", CoreV3GenImpl
# setupSyncWait) — Tile freely emits multi-wait instructions. Post-pass:
# move excess waits onto same-engine NOPs directly preceding the
# instruction; sequencer queues are FIFO, so this is equivalent.
_WAIT_LIMIT = 1


def _split_excess_waits(nc):
    f = nc.m.functions[0]
    for blk in f.blocks:
        out = []
        changed = False
        for inst in blk.instructions:
            si = inst.sync_info
            if si is not None and len(si.on_wait) > _WAIT_LIMIT:
                waits = list(si.on_wait)
                head, tail = waits[:-_WAIT_LIMIT], waits[-_WAIT_LIMIT:]
                for i in range(0, len(head), _WAIT_LIMIT):
                    out.append(
                        mybir.InstNoOp(
                            name=nc.get_next_instruction_name(),
                            sync_info=mybir.SyncInfo(
                                on_wait=head[i : i + _WAIT_LIMIT], on_update=[]
                            ),
                            bass_nofuse=True,
                            engine=inst.engine,
                        )
                    )
                inst.sync_info = mybir.SyncInfo(
                    on_wait=tail, on_update=list(si.on_update)
                )
                changed = True
            out.append(inst)
        if changed:
            blk.instructions = out


# ------------------------------------------------------------ bass program
def build_program(split_waits=True, phases=("qproj", "glu", "kproj", "scores")):
    nc = bass.Bass("TRN2", target_bir_lowering=False, debug=False)

    xT = nc.dram_tensor("xT", [BL, H, S], F32R, kind="ExternalInput")
    eT = nc.dram_tensor("eT", [H, BL * E], F32R, kind="ExternalInput")
    u_wT = nc.dram_tensor("u_wT", [H, H3], F32R, kind="ExternalInput")
    v_wT = nc.dram_tensor("v_wT", [H, H3], F32R, kind="ExternalInput")
    k_wT = nc.dram_tensor("k_wT", [H, H], F32R, kind="ExternalInput")
    q_wT = nc.dram_tensor("q_wT", [H, H], F32R, kind="ExternalInput")
    o_wT = nc.dram_tensor("o_wT", [H3, TAG], F32R, kind="ExternalInput")
    u_b = nc.dram_tensor("u_b", [H3], F32, kind="ExternalInput")
    v_b = nc.dram_tensor("v_b", [H3], F32, kind="ExternalInput")
    k_b = nc.dram_tensor("k_b", [H], F32, kind="ExternalInput")
    q_b = nc.dram_tensor("q_b", [H], F32, kind="ExternalInput")

    startT = nc.dram_tensor("startT", [BL, TAG, S], F32, kind="ExternalOutput")
    end_o = nc.dram_tensor("end_o", [BL, E, S], F32, kind="ExternalOutput")

    with tile.TileContext(nc) as tc, ExitStack() as ctx:
        const = ctx.enter_context(tc.tile_pool(name="const", bufs=1))

        o_sb = const.tile([128, NM, TAG], F32R)
        nc.sync.dma_start(o_sb[:], o_wT[:].rearrange("(m p) t -> p m t", p=128))
        ub_sb = const.tile([128, NM], F32)
        nc.sync.dma_start(ub_sb[:], u_b[:].rearrange("(m p) -> p m", p=128))
        vb_sb = const.tile([128, NM], F32)
        nc.sync.dma_start(vb_sb[:], v_b[:].rearrange("(m p) -> p m", p=128))
        kb_sb = const.tile([128, NK], F32)
        nc.sync.dma_start(kb_sb[:], k_b[:].rearrange("(m p) -> p m", p=128))
        qb_sb = const.tile([128, NK], F32)
        nc.sync.dma_start(qb_sb[:], q_b[:].rearrange("(m p) -> p m", p=128))
        eT_sb = const.tile([128, NK, BL * E], F32R)
        nc.sync.dma_start(eT_sb[:], eT[:].rearrange("(k p) f -> p k f", p=128))
        qT_sb = const.tile([128, NK, BL * E], F32R)

        # ---- q projection (both samples at once; N = BL*E = 256) --------
        if "qproj" in phases:
          with (
            tc.tile_pool(name="qw_pool", bufs=1) as qwp,
            tc.tile_pool(name="qp_psum", bufs=1, space="PSUM") as qpp,
        ):
            qw = qwp.tile([128, NK, H], F32R)
            nc.sync.dma_start(qw[:], q_wT[:].rearrange("(k p) o -> p k o", p=128))
            for m in range(NK):
                qp = qpp.tile([128, BL * E], F32, tag="qp", bufs=2)
                for k in range(NK):
                    nc.tensor.matmul(
                        qp[:],
                        qw[:, k, m * 128 : (m + 1) * 128],
                        eT_sb[:, k, :],
                        start=(k == 0),
                        stop=(k == NK - 1),
                    )
                nc.scalar.activation(
                    qT_sb[:, m, :], qp[:], ACT_F.Identity, bias=qb_sb[:, m : m + 1]
                )

        sample = ctx.enter_context(tc.tile_pool(name="sample", bufs=1))

        for s in range(BL):
            xt = sample.tile([128, NK, S], F32R, tag="xt", bufs=1, name=f"xt{s}")
            nc.sync.dma_start(xt[:], xT[s].rearrange("(k p) n -> p k n", p=128))

            # ---- GLU + start-logit projection ---------------------------
            if "glu" in phases:
              with (
                tc.tile_pool(name=f"glu_sb{s}", bufs=1) as gsb,
                tc.tile_pool(name=f"glu_ps{s}", bufs=1, space="PSUM") as gps,
            ):
                start_ps = gps.tile([TAG, S], F32, tag="start_ps", bufs=1)
                for m in range(NM):
                    uw = gsb.tile([128, NK, 128], F32R, tag="uw", bufs=2)
                    nc.sync.dma_start(
                        uw[:],
                        u_wT[:, m * 128 : (m + 1) * 128].rearrange(
                            "(k p) o -> p k o", p=128
                        ),
                    )
                    vw = gsb.tile([128, NK, 128], F32R, tag="vw", bufs=2)
                    nc.sync.dma_start(
                        vw[:],
                        v_wT[:, m * 128 : (m + 1) * 128].rearrange(
                            "(k p) o -> p k o", p=128
                        ),
                    )
                    for n in range(NN):
                        ns = slice(n * NCHUNK, (n + 1) * NCHUNK)
                        u_ps = gps.tile([128, NCHUNK], F32, tag="u_ps", bufs=2)
                        for k in range(NK):
                            nc.tensor.matmul(
                                u_ps[:],
                                uw[:, k, :],
                                xt[:, k, ns],
                                start=(k == 0),
                                stop=(k == NK - 1),
                            )
                        v_ps = gps.tile([128, NCHUNK], F32, tag="v_ps", bufs=2)
                        for k in range(NK):
                            nc.tensor.matmul(
                                v_ps[:],
                                vw[:, k, :],
                                xt[:, k, ns],
                                start=(k == 0),
                                stop=(k == NK - 1),
                            )
                        sig = gsb.tile([128, NCHUNK], F32, tag="sig", bufs=3)
                        nc.scalar.activation(
                            sig[:], u_ps[:], ACT_F.Sigmoid, bias=ub_sb[:, m : m + 1]
                        )
                        vbt = gsb.tile([128, NCHUNK], F32, tag="vbt", bufs=3)
                        nc.scalar.activation(
                            vbt[:], v_ps[:], ACT_F.Identity, bias=vb_sb[:, m : m + 1]
                        )
                        g = gsb.tile([128, NCHUNK], F32R, tag="g", bufs=3)
                        nc.vector.tensor_mul(g[:], sig[:], vbt[:])
                        nc.tensor.matmul(
                            start_ps[:, ns],
                            o_sb[:, m, :],
                            g[:],
                            start=(m == 0),
                            stop=(m == NM - 1),
                            skip_group_check=True,
                        )
                start_sb = gsb.tile([TAG, S], F32, tag="start_sb", bufs=1)
                nc.scalar.copy(start_sb[:], start_ps[:])
                nc.sync.dma_start(startT[s], start_sb[:])

            # ---- k projection ------------------------------------------
            kt = sample.tile([128, NK, S], F32R, tag="kt", bufs=1, name=f"kt{s}")
            if "kproj" in phases:
              with (
                tc.tile_pool(name=f"kw_sb{s}", bufs=1) as ksb,
                tc.tile_pool(name=f"kp_ps{s}", bufs=1, space="PSUM") as kps,
            ):
                kw = ksb.tile([128, NK, H], F32R, tag="kw", bufs=1)
                nc.sync.dma_start(kw[:], k_wT[:].rearrange("(k p) o -> p k o", p=128))
                for m in range(NK):
                    for n in range(NN):
                        ns = slice(n * NCHUNK, (n + 1) * NCHUNK)
                        kp = kps.tile([128, NCHUNK], F32, tag="kp", bufs=4)
                        for k in range(NK):
                            nc.tensor.matmul(
                                kp[:],
                                kw[:, k, m * 128 : (m + 1) * 128],
                                xt[:, k, ns],
                                start=(k == 0),
                                stop=(k == NK - 1),
                            )
                        nc.scalar.activation(
                            kt[:, m, ns], kp[:], ACT_F.Identity,
                            bias=kb_sb[:, m : m + 1],
                        )

            # ---- head-summed entity/sequence scores --------------------
            if "scores" in phases:
              with (
                tc.tile_pool(name=f"end_sb{s}", bufs=1) as esb,
                tc.tile_pool(name=f"sc_ps{s}", bufs=1, space="PSUM") as scps,
            ):
                # mean_h(q_h . k_h) / sqrt(D) == (1/(HEADS*sqrt(D))) * qT.T @ kT
                # -- the head sum is just the full H-dim contraction.
                end_sb = esb.tile([E, S], F32, tag="end_sb", bufs=1)
                for n in range(NN):
                    ns = slice(n * NCHUNK, (n + 1) * NCHUNK)
                    sc = scps.tile([E, NCHUNK], F32, tag="sc", bufs=4)
                    for k in range(NK):
                        nc.tensor.matmul(
                            sc[:],
                            qT_sb[:, k, s * E : (s + 1) * E],
                            kt[:, k, ns],
                            start=(k == 0),
                            stop=(k == NK - 1),
                        )
                    nc.scalar.mul(end_sb[:, ns], sc[:], OUT_SCALE)
                nc.sync.dma_start(end_o[s], end_sb[:])

    if split_waits:
        _split_excess_waits(nc)
    return nc


_PROGRAM = None


def _get_program():
    global _PROGRAM
    if _PROGRAM is None:
        _PROGRAM = build_program()
    return _PROGRAM


# ----------------------------------------------------------------- kernel
def kernel(hidden_state, u_w, u_b, v_w, v_b, o_w, o_b, q_w, q_b, k_w, k_b,
           entity_start, attention_mask):
    global LAST_EXEC_TIME_NS
    hidden = np.ascontiguousarray(np.asarray(hidden_state, dtype=np.float32))
    idx = np.asarray(entity_start).astype(np.int64)
    mask = np.asarray(attention_mask).astype(bool)

    # host-side layout work: transposes + the (FLOP-free) entity gather
    xT_all = np.ascontiguousarray(hidden.transpose(0, 2, 1))      # [B, H, S]
    ent = np.take_along_axis(hidden, idx[:, :, None], axis=1)     # [B, E, H]

    u_wT = np.ascontiguousarray(np.asarray(u_w, np.float32).T)
    v_wT = np.ascontiguousarray(np.asarray(v_w, np.float32).T)
    k_wT = np.ascontiguousarray(np.asarray(k_w, np.float32).T)
    q_wT = np.ascontiguousarray(np.asarray(q_w, np.float32).T)
    o_wT = np.ascontiguousarray(np.asarray(o_w, np.float32).T)
    ub = np.ascontiguousarray(np.asarray(u_b, np.float32))
    vb = np.ascontiguousarray(np.asarray(v_b, np.float32))
    kb = np.ascontiguousarray(np.asarray(k_b, np.float32))
    qb = np.ascontiguousarray(np.asarray(q_b, np.float32))

    in_maps = []
    for c in range(NCORES):
        sl = slice(c * BL, (c + 1) * BL)
        eTc = np.ascontiguousarray(
            ent[sl].transpose(2, 0, 1).reshape(H, BL * E)
        )
        in_maps.append({
            "xT": np.ascontiguousarray(xT_all[sl]),
            "eT": eTc,
            "u_wT": u_wT, "v_wT": v_wT, "k_wT": k_wT, "q_wT": q_wT,
            "o_wT": o_wT,
            "u_b": ub, "v_b": vb, "k_b": kb, "q_b": qb,
        })

    nc = _get_program()
    kwargs = {}
    if PROFILE:
        import tempfile
        import concourse.bass_utils as _bu
        _bu.upload_artifacts = lambda d: d     # keep NTFF artifacts local
        kwargs["trace"] = True
        kwargs["tmpdir"] = tempfile.mkdtemp(prefix="ner_trace_")
    res = run_bass_kernel_spmd(nc, in_maps, core_ids=list(range(NCORES)), **kwargs)
    LAST_EXEC_TIME_NS = res.exec_time_ns

    start_t = np.concatenate(
        [res.results[c]["startT"] for c in range(NCORES)], axis=0
    )                                                             # [B, TAG, S]
    end_v = np.concatenate(
        [res.results[c]["end_o"] for c in range(NCORES)], axis=0
    )                                                             # [B, E, S]

    start_logit = start_t.transpose(0, 2, 1) + np.asarray(o_b, np.float32)[None, None, :]
    end_logit = np.where(mask[:, None, :], end_v, np.float32(-50000.0))
    return (
        np.ascontiguousarray(start_logit, dtype=np.float32),
        np.ascontiguousarray(end_logit, dtype=np.float32),
    )


# revision 15
# speedup vs baseline: 1.0375x; 1.0375x over previous
"""Trainium2 Bass kernel for nn_NER_76012331205088.

Reference computation (per batch sample b):
    u = hidden @ u_w.T + u_b                  # [S, 3H]
    v = hidden @ v_w.T + v_b                  # [S, 3H]
    start_logit = (sigmoid(u) * v) @ o_w.T + o_b          # [S, TAG]
    entity = hidden[entity_start]             # [E, H]
    q = (entity @ q_w.T + q_b)  -> [E, HEADS, D]
    k = (hidden @ k_w.T + k_b)  -> [S, HEADS, D]
    end_logit = mean_h(q_h @ k_h.T) / sqrt(D), masked_fill(~mask, -5e4)

Sharding: pure data-parallel over batch B=16 across 8 cores (2 samples per
core); weights replicated to every core; no collectives — the host slices
inputs and concatenates outputs.

Device layout strategy: every matmul contracts along the SBUF partition
dim, so everything is computed in "transposed" space:
    uT[o, s] = sum_h u_wT[h, o] * xT[h, s]
with xT/weights pre-transposed once on the host (pure layout work, no
FLOPs). Matmuls run as float32r (fp32 bits, single-pass PE mode: 1
cycle/row at N>=256 vs 4 for plain fp32). PSUM accumulates in fp32.

The GLU o-projection accumulates start_logit.T directly in a persistent
[TAG, S] PSUM region across all 18 o-tiles (sigmoid(u)*v tiles are consumed
immediately; the [3H, S] GLU activation never materializes). Head-summed
scores accumulate over the 12 heads in PSUM, with the 1/(HEADS*sqrt(D))
scale folded into the PSUM->SBUF copy.

Biases u_b/v_b/k_b/q_b are applied on-device (fused into ACT ops); o_b and
the attention-mask fill are applied on the host (o_b is a rank-1 post-add,
the mask is elementwise on the output).
"""

import os
import sys
from contextlib import ExitStack

import numpy as np

for _p in (
    "/root/.axon_site",
    "/root/.axon_site/_ro/trn_rl_repo",
    "/root/.axon_site/_ro/pypackages",
    "/opt/trn_rl_repo",
):
    if os.path.isdir(_p) and _p not in sys.path:
        sys.path.append(_p)

import concourse.bass as bass
import concourse.mybir as mybir
import concourse.tile as tile
from concourse.bass_utils import run_bass_kernel_spmd
from concourse.vector_clock import ScopedClock

# ---------------------------------------------------------------- constants
B, S, H, E, TAG, HEADS = 16, 2048, 768, 128, 10, 12
D = H // HEADS          # 64
H3 = 3 * H              # 2304
NCORES = 8
BL = B // NCORES        # 2 samples per core
NCHUNK = 512            # free-dim chunk (one fp32 PSUM bank)
NK = H // 128           # 6 contraction tiles over H
NM = H3 // 128          # 18 output tiles over 3H
NN = S // NCHUNK        # 4 sequence chunks
OUT_SCALE = 1.0 / (HEADS * float(D) ** 0.5)   # mean over heads + /sqrt(D)

F32 = mybir.dt.float32
F32R = mybir.dt.float32r
ACT_F = mybir.ActivationFunctionType

PROFILE = False         # set True (module level) to trace + report HW time
LAST_EXEC_TIME_NS = None


# ---------------------------------------------------- wait-split post-pass
# The walrus build in this container rejects instructions carrying more
# than one sem wait ("Too many sync wait commands", CoreV3GenImpl
# setupSyncWait) — Tile freely emits multi-wait instructions. Post-pass:
# move excess waits onto same-engine NOPs directly preceding the
# instruction; sequencer queues are FIFO, so this is equivalent.
_WAIT_LIMIT = 1


def _split_excess_waits(nc):
    f = nc.m.functions[0]
    for blk in f.blocks:
        out = []
        changed = False
        for inst in blk.instructions:
            si = inst.sync_info
            if si is not None and len(si.on_wait) > _WAIT_LIMIT:
                waits = list(si.on_wait)
                head, tail = waits[:-_WAIT_LIMIT], waits[-_WAIT_LIMIT:]
                for i in range(0, len(head), _WAIT_LIMIT):
                    out.append(
                        mybir.InstNoOp(
                            name=nc.get_next_instruction_name(),
                            sync_info=mybir.SyncInfo(
                                on_wait=head[i : i + _WAIT_LIMIT], on_update=[]
                            ),
                            bass_nofuse=True,
                            engine=inst.engine,
                        )
                    )
                inst.sync_info = mybir.SyncInfo(
                    on_wait=tail, on_update=list(si.on_update)
                )
                changed = True
            out.append(inst)
        if changed:
            blk.instructions = out


# ------------------------------------------------------------ bass program
def build_program(split_waits=True, phases=("qproj", "glu", "kproj", "scores")):
    nc = bass.Bass("TRN2", target_bir_lowering=False, debug=False)

    xT = nc.dram_tensor("xT", [BL, H, S], F32R, kind="ExternalInput")
    eT = nc.dram_tensor("eT", [H, BL * E], F32R, kind="ExternalInput")
    u_wT = nc.dram_tensor("u_wT", [H, H3], F32R, kind="ExternalInput")
    v_wT = nc.dram_tensor("v_wT", [H, H3], F32R, kind="ExternalInput")
    k_wT = nc.dram_tensor("k_wT", [H, H], F32R, kind="ExternalInput")
    q_wT = nc.dram_tensor("q_wT", [H, H], F32R, kind="ExternalInput")
    o_wT = nc.dram_tensor("o_wT", [H3, TAG], F32R, kind="ExternalInput")
    u_b = nc.dram_tensor("u_b", [H3], F32, kind="ExternalInput")
    v_b = nc.dram_tensor("v_b", [H3], F32, kind="ExternalInput")
    k_b = nc.dram_tensor("k_b", [H], F32, kind="ExternalInput")
    q_b = nc.dram_tensor("q_b", [H], F32, kind="ExternalInput")

    startT = nc.dram_tensor("startT", [BL, TAG, S], F32, kind="ExternalOutput")
    end_o = nc.dram_tensor("end_o", [BL, E, S], F32, kind="ExternalOutput")

    with tile.TileContext(nc) as tc, ExitStack() as ctx:
        const = ctx.enter_context(tc.tile_pool(name="const", bufs=1))

        o_sb = const.tile([128, NM, TAG], F32R)
        nc.sync.dma_start(o_sb[:], o_wT[:].rearrange("(m p) t -> p m t", p=128))
        ub_sb = const.tile([128, NM], F32)
        nc.sync.dma_start(ub_sb[:], u_b[:].rearrange("(m p) -> p m", p=128))
        vb_sb = const.tile([128, NM], F32)
        nc.sync.dma_start(vb_sb[:], v_b[:].rearrange("(m p) -> p m", p=128))
        kb_sb = const.tile([128, NK], F32)
        nc.sync.dma_start(kb_sb[:], k_b[:].rearrange("(m p) -> p m", p=128))
        qb_sb = const.tile([128, NK], F32)
        nc.sync.dma_start(qb_sb[:], q_b[:].rearrange("(m p) -> p m", p=128))
        eT_sb = const.tile([128, NK, BL * E], F32R)
        nc.sync.dma_start(eT_sb[:], eT[:].rearrange("(k p) f -> p k f", p=128))
        qT_sb = const.tile([128, NK, BL * E], F32R)

        # ---- q projection (both samples at once; N = BL*E = 256) --------
        if "qproj" in phases:
          with (
            tc.tile_pool(name="qw_pool", bufs=1) as qwp,
            tc.tile_pool(name="qp_psum", bufs=1, space="PSUM") as qpp,
        ):
            qw = qwp.tile([128, NK, H], F32R)
            nc.sync.dma_start(qw[:], q_wT[:].rearrange("(k p) o -> p k o", p=128))
            for m in range(NK):
                qp = qpp.tile([128, BL * E], F32, tag="qp", bufs=2)
                for k in range(NK):
                    nc.tensor.matmul(
                        qp[:],
                        qw[:, k, m * 128 : (m + 1) * 128],
                        eT_sb[:, k, :],
                        start=(k == 0),
                        stop=(k == NK - 1),
                    )
                nc.scalar.activation(
                    qT_sb[:, m, :], qp[:], ACT_F.Identity, bias=qb_sb[:, m : m + 1]
                )

        sample = ctx.enter_context(tc.tile_pool(name="sample", bufs=1))

        for s in range(BL):
            # bufs=2: next sample's activations prefetch under this one's compute
            xt = sample.tile([128, NK, S], F32R, tag="xt", bufs=2, name=f"xt{s}")
            nc.sync.dma_start(xt[:], xT[s].rearrange("(k p) n -> p k n", p=128))

            # ---- GLU + start-logit projection ---------------------------
            if "glu" in phases:
              with (
                tc.tile_pool(name=f"glu_sb{s}", bufs=1) as gsb,
                tc.tile_pool(name=f"glu_ps{s}", bufs=1, space="PSUM") as gps,
            ):
                start_ps = gps.tile([TAG, S], F32, tag="start_ps", bufs=1)
                for m in range(NM):
                    uw = gsb.tile([128, NK, 128], F32R, tag="uw", bufs=2)
                    nc.sync.dma_start(
                        uw[:],
                        u_wT[:, m * 128 : (m + 1) * 128].rearrange(
                            "(k p) o -> p k o", p=128
                        ),
                    )
                    vw = gsb.tile([128, NK, 128], F32R, tag="vw", bufs=2)
                    nc.sync.dma_start(
                        vw[:],
                        v_wT[:, m * 128 : (m + 1) * 128].rearrange(
                            "(k p) o -> p k o", p=128
                        ),
                    )
                    for n in range(NN):
                        ns = slice(n * NCHUNK, (n + 1) * NCHUNK)
                        u_ps = gps.tile([128, NCHUNK], F32, tag="u_ps", bufs=2)
                        for k in range(NK):
                            nc.tensor.matmul(
                                u_ps[:],
                                uw[:, k, :],
                                xt[:, k, ns],
                                start=(k == 0),
                                stop=(k == NK - 1),
                            )
                        v_ps = gps.tile([128, NCHUNK], F32, tag="v_ps", bufs=2)
                        for k in range(NK):
                            nc.tensor.matmul(
                                v_ps[:],
                                vw[:, k, :],
                                xt[:, k, ns],
                                start=(k == 0),
                                stop=(k == NK - 1),
                            )
                        sig = gsb.tile([128, NCHUNK], F32, tag="sig", bufs=3)
                        nc.scalar.activation(
                            sig[:], u_ps[:], ACT_F.Sigmoid, bias=ub_sb[:, m : m + 1]
                        )
                        g = gsb.tile([128, NCHUNK], F32R, tag="g", bufs=3)
                        # g = (v + v_b) * sigmoid(u + u_b), one DVE op
                        nc.vector.scalar_tensor_tensor(
                            g[:], v_ps[:], vb_sb[:, m : m + 1], sig[:],
                            mybir.AluOpType.add, mybir.AluOpType.mult,
                        )
                        nc.tensor.matmul(
                            start_ps[:, ns],
                            o_sb[:, m, :],
                            g[:],
                            start=(m == 0),
                            stop=(m == NM - 1),
                            skip_group_check=True,
                        )
                start_sb = gsb.tile([TAG, S], F32, tag="start_sb", bufs=1)
                nc.scalar.copy(start_sb[:], start_ps[:])
                nc.sync.dma_start(startT[s], start_sb[:])

            # ---- k projection + scores, fused per sequence chunk --------
            # mean_h(q_h . k_h) / sqrt(D) == (1/(HEADS*sqrt(D))) * qT.T @ kT
            # -- the head sum is just the full H-dim contraction, so kT for
            # one chunk is consumed immediately and never fully materialized.
            if "kproj" in phases and "scores" in phases:
              with (
                tc.tile_pool(name=f"kw_sb{s}", bufs=1) as ksb,
                tc.tile_pool(name=f"kp_ps{s}", bufs=1, space="PSUM") as kps,
            ):
                kw = ksb.tile([128, NK, H], F32R, tag="kw", bufs=1)
                nc.sync.dma_start(kw[:], k_wT[:].rearrange("(k p) o -> p k o", p=128))
                end_sb = ksb.tile([E, S], F32, tag="end_sb", bufs=1)
                for n in range(NN):
                    ns = slice(n * NCHUNK, (n + 1) * NCHUNK)
                    ktn = ksb.tile([128, NK, NCHUNK], F32R, tag="ktn", bufs=2)
                    for m in range(NK):
                        kp = kps.tile([128, NCHUNK], F32, tag="kp", bufs=3)
                        for k in range(NK):
                            nc.tensor.matmul(
                                kp[:],
                                kw[:, k, m * 128 : (m + 1) * 128],
                                xt[:, k, ns],
                                start=(k == 0),
                                stop=(k == NK - 1),
                            )
                        nc.scalar.activation(
                            ktn[:, m, :], kp[:], ACT_F.Identity,
                            bias=kb_sb[:, m : m + 1],
                        )
                    sc = kps.tile([E, NCHUNK], F32, tag="sc", bufs=2)
                    for k in range(NK):
                        nc.tensor.matmul(
                            sc[:],
                            qT_sb[:, k, s * E : (s + 1) * E],
                            ktn[:, k, :],
                            start=(k == 0),
                            stop=(k == NK - 1),
                        )
                    nc.scalar.mul(end_sb[:, ns], sc[:], OUT_SCALE)
                nc.sync.dma_start(end_o[s], end_sb[:])

    if split_waits:
        _split_excess_waits(nc)
    return nc


_PROGRAM = None


def _get_program():
    global _PROGRAM
    if _PROGRAM is None:
        _PROGRAM = build_program()
    return _PROGRAM


# ----------------------------------------------------------------- kernel
def kernel(hidden_state, u_w, u_b, v_w, v_b, o_w, o_b, q_w, q_b, k_w, k_b,
           entity_start, attention_mask):
    global LAST_EXEC_TIME_NS
    hidden = np.ascontiguousarray(np.asarray(hidden_state, dtype=np.float32))
    idx = np.asarray(entity_start).astype(np.int64)
    mask = np.asarray(attention_mask).astype(bool)

    # host-side layout work: transposes + the (FLOP-free) entity gather
    xT_all = np.ascontiguousarray(hidden.transpose(0, 2, 1))      # [B, H, S]
    ent = np.take_along_axis(hidden, idx[:, :, None], axis=1)     # [B, E, H]

    u_wT = np.ascontiguousarray(np.asarray(u_w, np.float32).T)
    v_wT = np.ascontiguousarray(np.asarray(v_w, np.float32).T)
    k_wT = np.ascontiguousarray(np.asarray(k_w, np.float32).T)
    q_wT = np.ascontiguousarray(np.asarray(q_w, np.float32).T)
    o_wT = np.ascontiguousarray(np.asarray(o_w, np.float32).T)
    ub = np.ascontiguousarray(np.asarray(u_b, np.float32))
    vb = np.ascontiguousarray(np.asarray(v_b, np.float32))
    kb = np.ascontiguousarray(np.asarray(k_b, np.float32))
    qb = np.ascontiguousarray(np.asarray(q_b, np.float32))

    in_maps = []
    for c in range(NCORES):
        sl = slice(c * BL, (c + 1) * BL)
        eTc = np.ascontiguousarray(
            ent[sl].transpose(2, 0, 1).reshape(H, BL * E)
        )
        in_maps.append({
            "xT": np.ascontiguousarray(xT_all[sl]),
            "eT": eTc,
            "u_wT": u_wT, "v_wT": v_wT, "k_wT": k_wT, "q_wT": q_wT,
            "o_wT": o_wT,
            "u_b": ub, "v_b": vb, "k_b": kb, "q_b": qb,
        })

    nc = _get_program()
    kwargs = {}
    if PROFILE:
        import tempfile
        import concourse.bass_utils as _bu
        _bu.upload_artifacts = lambda d: d     # keep NTFF artifacts local
        kwargs["trace"] = True
        kwargs["tmpdir"] = tempfile.mkdtemp(prefix="ner_trace_")
    res = run_bass_kernel_spmd(nc, in_maps, core_ids=list(range(NCORES)), **kwargs)
    LAST_EXEC_TIME_NS = res.exec_time_ns

    start_t = np.concatenate(
        [res.results[c]["startT"] for c in range(NCORES)], axis=0
    )                                                             # [B, TAG, S]
    end_v = np.concatenate(
        [res.results[c]["end_o"] for c in range(NCORES)], axis=0
    )                                                             # [B, E, S]

    start_logit = start_t.transpose(0, 2, 1) + np.asarray(o_b, np.float32)[None, None, :]
    end_logit = np.where(mask[:, None, :], end_v, np.float32(-50000.0))
    return (
        np.ascontiguousarray(start_logit, dtype=np.float32),
        np.ascontiguousarray(end_logit, dtype=np.float32),
    )
